# revision 10
# baseline (speedup 1.0000x reference)
"""Trainium2 Bass kernel for AEConv9 (9x9 conv autoencoder with top-1 channel
masking), data-parallel over batch across 8 NeuronCores.

Self-contained: hardcodes shapes from the problem spec
(x [16,1,128,128] f32, 256 channels, 128 latent, 9x9 kernels, pad 4).
"""

import sys

sys.path.insert(0, "/opt/trn_rl_repo")

import numpy as np

import concourse.bass as bass
import concourse.mybir as mybir
import concourse.tile as tile
from concourse.ap import AP
from concourse.masks import make_identity
from concourse.bass_utils import run_bass_kernel_spmd

F32 = mybir.dt.float32
F32R = mybir.dt.float32r
BF16 = mybir.dt.bfloat16
U32 = mybir.dt.uint32

N_CORES = 8
B = 16
BS = B // N_CORES  # images per core
C = 256  # channels
D = 128  # latent
H = W = 128
HP, WP = 136, 144  # padded image: 4 rows top/bottom, 4 cols left, 12 right
HPS = HP + 1  # +1 spare row so the flat 576-wide im2col window stays in bounds
NYS = HPS * WP
NCHUNK_PER_IMG = 32  # 4 rows of 128 px per chunk
F = 512  # pixels per chunk
NCHUNK = BS * NCHUNK_PER_IMG
RELU = mybir.ActivationFunctionType.Relu
SIGMOID = mybir.ActivationFunctionType.Sigmoid
COPYF = mybir.ActivationFunctionType.Copy


# ---------------------------------------------------------------- wait fixing
def _max_inline_waits(inst) -> int:
    return 0


def fix_waits(nc):
    """Hoist inline sem waits into standalone event-semaphore nops on the same
    engine queue (this walrus build supports ~0-1 inline waits per inst)."""
    counter = [0]

    def make_wait_nop(engine, wait):
        counter[0] += 1
        nop = mybir.InstEventSemaphore(name=f"WFIX-{counter[0]}")
        nop.engine = engine
        nop.sync_info = mybir.SyncInfo(on_wait=[wait], on_update=[])
        nc.register_instruction(nop, overwrite=True)
        return nop

    for f in nc.m.functions:
        for bb in f.blocks:
            insts = list(bb.instructions)
            out = []
            changed = False
            for inst in insts:
                si = inst.sync_info
                waits = list(si.on_wait) if (si and si.on_wait) else []
                keep = _max_inline_waits(inst)
                if len(waits) > keep:
                    changed = True
                    excess = waits[: len(waits) - keep]
                    kept = waits[len(waits) - keep :]
                    for w in excess:
                        out.append(make_wait_nop(inst.engine, w))
                    inst.sync_info.on_wait = kept
                out.append(inst)
            if changed:
                bb.instructions[:] = out
    return nc


# ---------------------------------------------------------------- builder
def build_nc(nchunk=NCHUNK):
    nc = bass.Bass(target_bir_lowering=False)

    xpad = nc.declare_dram_parameter("xpad", [BS, HPS, WP], F32, isOutput=False)
    wenc_d = nc.declare_dram_parameter("wenc", [81, C], F32, isOutput=False)
    wdn_d = nc.declare_dram_parameter("wdn", [C, D], F32, isOutput=False)
    wup_d = nc.declare_dram_parameter("wup", [D, C], F32, isOutput=False)
    wdec_d = nc.declare_dram_parameter("wdec", [C, 81], F32, isOutput=False)
    gate_d = nc.declare_dram_parameter("gate", [C], F32, isOutput=False)
    bdn_d = nc.declare_dram_parameter("bdn", [D], F32, isOutput=False)
    bup_d = nc.declare_dram_parameter("bup", [C], F32, isOutput=False)
    iota_d = nc.declare_dram_parameter("iota", [C], F32, isOutput=False)
    ema99_d = nc.declare_dram_parameter("ema99", [C], F32, isOutput=False)

    xhat_d = nc.declare_dram_parameter("xhat", [BS * H * W], F32, isOutput=True)
    usage_d = nc.declare_dram_parameter("usage", [C], F32, isOutput=True)

    # p scale: matches ref p = cnt / (B*H*W + 1e-6); usage = ema*0.99 + 0.01*p
    p_scale = float(np.float32(0.01) / np.float32(B * H * W + 1e-6))

    with tile.TileContext(nc) as tc:
        with (
            tc.tile_pool(name="const", bufs=1) as cpool,
            tc.tile_pool(name="work", bufs=1) as wpool,
            tc.tile_pool(name="psum", bufs=1, space="PSUM") as psp,
            tc.tile_pool(name="dram", bufs=1, space="DRAM") as dpool,
        ):
            # ---------------- constants / weights ----------------
            wenc = cpool.tile([81, C], F32, name="wenc")
            nc.sync.dma_start(wenc[:, :], wenc_d[:, :])

            # staging for f32r casts
            stg = cpool.tile([128, C], F32, name="stg")

            wdn = [cpool.tile([128, D], F32R, name=f"wdn{h}") for h in range(2)]
            nc.sync.dma_start(stg[:, 0:D], wdn_d[0:128, :])
            nc.vector.tensor_copy(wdn[0][:, :], stg[:, 0:D])
            nc.sync.dma_start(stg[:, D : 2 * D], wdn_d[128:256, :])
            nc.vector.tensor_copy(wdn[1][:, :], stg[:, D : 2 * D])

            wup = cpool.tile([D, C], F32R, name="wup")
            nc.sync.dma_start(stg[:, 0:C], wup_d[:, :])
            nc.vector.tensor_copy(wup[:, :], stg[:, 0:C])

            wdec = [cpool.tile([128, 81], F32R, name=f"wdec{h}") for h in range(2)]
            stg2 = cpool.tile([128, 192], F32, name="stg2")
            nc.sync.dma_start(stg2[:, 0:81], wdec_d[0:128, :])
            nc.vector.tensor_copy(wdec[0][:, :], stg2[:, 0:81])
            nc.sync.dma_start(stg2[:, 96 : 96 + 81], wdec_d[128:256, :])
            nc.vector.tensor_copy(wdec[1][:, :], stg2[:, 96 : 96 + 81])

            # small [128, 2] constants: col h = values for channels h*128..h*128+127
            def load_col2(dram):
                t = cpool.tile([128, 2], F32, name=f"cc_{dram.name}")
                src = AP(dram.ap().tensor, 0, [[1, 128], [128, 2]])
                nc.sync.dma_start(t[:, :], src)
                return t

            gate_t = load_col2(gate_d)
            iota_t = load_col2(iota_d)
            ema99_t = load_col2(ema99_d)
            bup_t = load_col2(bup_d)
            bdn_t = cpool.tile([128, 1], F32, name="bdn_t")
            nc.sync.dma_start(bdn_t[:, :], AP(bdn_d.ap().tensor, 0, [[1, 128], [128, 1]]))

            sig_t = cpool.tile([128, 2], F32, name="sig_t")
            nc.scalar.activation(sig_t[:, :], gate_t[:, :], SIGMOID)

            ident_f = cpool.tile([128, 128], F32, name="ident_f")
            make_identity(nc, ident_f[:, :])
            ident_b = cpool.tile([128, 128], BF16, name="ident_b")
            make_identity(nc, ident_b[:, :])
            ones_b = cpool.tile([1, 128], BF16, name="ones_b")
            nc.gpsimd.memset(ones_b[:, :], 1.0)
            ones81 = cpool.tile([81, 1], F32R, name="ones81")
            ones81f = cpool.tile([81, 1], F32, name="ones81f")
            nc.gpsimd.memset(ones81f[:, :], 1.0)
            nc.vector.tensor_copy(ones81[:, :], ones81f[:, :])

            # p-count accumulators
            pcnt = [cpool.tile([128, NCHUNK], F32, name=f"pcnt{h}") for h in range(2)]

            # DRAM Y staging, one per image (avoids cross-image WAR serialization)
            YSd = [dpool.tile([81, NYS], F32R, name=f"YSd{b}") for b in range(BS)]
            zero_sb = cpool.tile([81, 1536], F32R, name="zero_sb")
            zero_f = cpool.tile([81, 1536], F32, name="zero_f")
            nc.gpsimd.memset(zero_f[:, :], 0.0)
            nc.vector.tensor_copy(zero_sb[:, :], zero_f[:, :])
            for b in range(BS):
                ys3 = YSd[b][:, :].rearrange("p (y x) -> p y x", x=WP)
                # top 4 + bottom 4 (+1 spare) rows
                nc.sync.dma_start(ys3[:, 0:4, :], zero_sb[:, 0 : 4 * WP])
                nc.sync.dma_start(ys3[:, HP - 4 : HPS, :], zero_sb[:, 0 : 5 * WP])
                # left 4 and right 12 cols of interior rows
                nc.sync.dma_start(ys3[:, 4 : HP - 4, 0:4], zero_sb[:, 0 : 4 * (HP - 8)])
                nc.sync.dma_start(
                    ys3[:, 4 : HP - 4, WP - 12 : WP], zero_sb[:, 0 : 12 * (HP - 8)]
                )

            # ---------------- per-chunk pipeline ----------------
            state = {}

            def front(f):
                b, yb = divmod(f, NCHUNK_PER_IMG)
                y0 = 4 * yb
                # im2col load: XC[t=(dy,dx), q] = xpad[b, y0+dy, dx+q] flattened
                XC = wpool.tile([81, 576], F32, name="XC", bufs=3)
                src = AP(
                    xpad.ap().tensor,
                    b * NYS + y0 * WP,
                    [[WP, 9], [1, 9], [1, 576]],
                )
                nc.sync.dma_start(XC[:, :], src)
                xc3 = XC[:, :].rearrange("p (y x) -> p y x", x=WP)[:, :, 0:128]

                # encoder fp32 matmuls + gating
                A = []
                for h in range(2):
                    Z = psp.tile([128, F], F32, name=f"pz{h}", tag="pz", bufs=2)
                    nc.tensor.matmul(
                        Z[:, :].rearrange("p (y x) -> p y x", x=128),
                        wenc[:, h * 128 : (h + 1) * 128],
                        xc3,
                        start=True,
                        stop=True,
                    )
                    A_h = wpool.tile([128, F], F32, name=f"A{h}", bufs=2)
                    nc.scalar.activation(
                        A_h[:, :], Z[:, :], RELU, scale=sig_t[:, h : h + 1]
                    )
                    A.append(A_h)

                # per-subchunk argmax
                irow = psp.tile([1, F], F32, name="irow", tag="pmisc", bufs=2)
                for s in range(4):
                    TP = psp.tile([128, 256], F32, name="TP", tag="ptp", bufs=1)
                    for h in range(2):
                        nc.tensor.transpose(
                            TP[:, h * 128 : (h + 1) * 128],
                            A[h][:, s * 128 : (s + 1) * 128],
                            ident_f[:, :],
                        )
                    mx8 = wpool.tile([128, 8], F32, name="mx8", bufs=4)
                    idx8 = wpool.tile([128, 8], U32, name="idx8", bufs=4)
                    nc.vector.max(mx8[:, :], TP[:, :])
                    nc.vector.max_index(idx8[:, :], mx8[:, :], TP[:, :])
                    idxf = wpool.tile([128, 1], F32, name="idxf", bufs=4)
                    nc.vector.tensor_copy(idxf[:, :], idx8[:, 0:1])
                    nc.tensor.transpose(
                        irow[0:1, s * 128 : (s + 1) * 128], idxf[:, :], ident_f[:, :]
                    )
                irow_sb = wpool.tile([1, F], BF16, name="irow_sb", bufs=2)
                nc.vector.tensor_copy(irow_sb[:, :], irow[:, :])
                IDXB = psp.tile([128, F], F32, name="IDXB", tag="pmisc", bufs=2)
                nc.tensor.matmul(
                    IDXB[:, :], ones_b[:, :], irow_sb[:, :], start=True, stop=True
                )

                # compare (counts) + fused apply -> masked activations in f32r
                Am = []
                for h in range(2):
                    mask_h = wpool.tile([128, F], BF16, name=f"mask{h}", bufs=2)
                    nc.vector.tensor_scalar(
                        out=mask_h[:, :],
                        in0=IDXB[:, :],
                        scalar1=iota_t[:, h : h + 1],
                        scalar2=0.0,
                        op0=mybir.AluOpType.is_equal,
                        op1=mybir.AluOpType.add,
                        accum_out=pcnt[h][:, f : f + 1],
                    )
                    Am_h = wpool.tile([128, F], F32R, name=f"Am{h}", bufs=2)
                    nc.vector.scalar_tensor_tensor(
                        out=Am_h[:, :],
                        in0=IDXB[:, :],
                        scalar=iota_t[:, h : h + 1],
                        in1=A[h][:, :],
                        op0=mybir.AluOpType.is_equal,
                        op1=mybir.AluOpType.mult,
                    )
                    Am.append(Am_h)

                # down projection (f32r)
                Hp = psp.tile([128, F], F32, name="Hp", tag="pmm", bufs=2)
                for h in range(2):
                    nc.tensor.matmul(
                        Hp[:, :], wdn[h][:, :], Am[h][:, :], start=(h == 0), stop=(h == 1)
                    )
                h_sb = wpool.tile([128, F], F32R, name="h_sb", bufs=2)
                nc.scalar.activation(h_sb[:, :], Hp[:, :], RELU, bias=bdn_t[:, :])

                # up projection
                A2 = []
                for h in range(2):
                    A2p = psp.tile([128, F], F32, name=f"A2p{h}", tag="pmm", bufs=2)
                    nc.tensor.matmul(
                        A2p[:, :], wup[:, h * 128 : (h + 1) * 128], h_sb[:, :],
                        start=True, stop=True,
                    )
                    a2_h = wpool.tile([128, F], F32R, name=f"a2_{h}", bufs=2)
                    nc.scalar.activation(
                        a2_h[:, :], A2p[:, :], RELU, bias=bup_t[:, h : h + 1]
                    )
                    A2.append(a2_h)

                # decoder Y = wdec^T @ a2  [81, F]
                Yp = psp.tile([81, F], F32, name="Yp", tag="pY", bufs=1)
                for h in range(2):
                    nc.tensor.matmul(
                        Yp[:, :], wdec[h][:, :], A2[h][:, :], start=(h == 0), stop=(h == 1)
                    )
                Y_sb = wpool.tile([81, F], F32R, name="Y_sb", bufs=2)
                nc.scalar.copy(Y_sb[:, :], Yp[:, :])
                ys3 = YSd[b][:, :].rearrange("p (y x) -> p y x", x=WP)
                nc.sync.dma_start(ys3[:, y0 + 4 : y0 + 8, 4:132], Y_sb[:, :])

            def back(f):
                b, yb = divmod(f, NCHUNK_PER_IMG)
                y0 = 4 * yb
                G = wpool.tile([81, 576], F32R, name="G", bufs=3)
                gsrc = AP(
                    YSd[b][:, :].tensor,
                    y0 * WP,
                    [[NYS * 9 + WP, 9], [NYS + 1, 9], [1, 576]],
                )
                nc.sync.dma_start(G[:, :], gsrc)
                XHp = psp.tile([1, F], F32, name="XHp", tag="pmisc", bufs=2)
                g3 = G[:, :].rearrange("p (y x) -> p y x", x=WP)[:, :, 0:128]
                nc.tensor.matmul(
                    XHp[0:1, :].rearrange("p (y x) -> p y x", x=128),
                    ones81[:, :],
                    g3,
                    start=True,
                    stop=True,
                )
                xh_sb = wpool.tile([1, F], F32, name="xh_sb", bufs=2)
                nc.vector.tensor_copy(xh_sb[:, :], XHp[:, :])
                nc.sync.dma_start(xhat_d[f * F : (f + 1) * F], xh_sb[0:1, :])

            for f in range(nchunk):
                front(f)
                if f >= 1:
                    back(f - 1)
            back(nchunk - 1)

            # ---------------- usage EMA with AllReduce ----------------
            cnt_sb = cpool.tile([128, 2], F32, name="cnt_sb")
            for h in range(2):
                nc.vector.reduce_sum(
                    cnt_sb[:, h : h + 1], pcnt[h][:, :], axis=mybir.AxisListType.X,
                    op=mybir.AluOpType.add,
                )
            pb_in = dpool.tile([C], F32, name="pb_in")
            pb_out = dpool.tile([C], F32, name="pb_out", addr_space="Shared")
            dst = AP(pb_in[:].tensor, 0, [[1, 128], [128, 2]])
            nc.gpsimd.dma_start(dst, cnt_sb[:, :])
            nc.gpsimd.collective_compute(
                "AllReduce",
                mybir.AluOpType.add,
                ins=[pb_in[:].opt()],
                outs=[pb_out[:].opt()],
                replica_groups=[list(range(N_CORES))],
            )
            cnt_g = cpool.tile([128, 2], F32, name="cnt_g")
            nc.gpsimd.dma_start(cnt_g[:, :], AP(pb_out[:].tensor, 0, [[1, 128], [128, 2]]))
            usage_sb = cpool.tile([128, 2], F32, name="usage_sb")
            nc.vector.scalar_tensor_tensor(
                out=usage_sb[:, :],
                in0=cnt_g[:, :],
                scalar=p_scale,
                in1=ema99_t[:, :],
                op0=mybir.AluOpType.mult,
                op1=mybir.AluOpType.add,
            )
            nc.gpsimd.dma_start(
                AP(usage_d.ap().tensor, 0, [[1, 128], [128, 2]]), usage_sb[:, :]
            )

    fix_waits(nc)
    return nc


_NC_CACHE = {}


def make_in_maps(x, w_conv, gate_logit, w_down, b_down, w_up, b_up, usage_ema):
    x = np.asarray(x, dtype=np.float32)
    w_conv = np.asarray(w_conv, dtype=np.float32)
    gate_logit = np.asarray(gate_logit, dtype=np.float32)
    w_down = np.asarray(w_down, dtype=np.float32)
    b_down = np.asarray(b_down, dtype=np.float32)
    w_up = np.asarray(w_up, dtype=np.float32)
    b_up = np.asarray(b_up, dtype=np.float32)
    usage_ema = np.asarray(usage_ema, dtype=np.float32)

    wenc = w_conv.reshape(C, 81).T.copy()  # [81, C], t = dy*9+dx
    wdn = w_down.T.copy()  # [C, D]
    wup = w_up.T.copy()  # [D, C]
    wdec = np.flip(w_conv[:, 0], axis=(1, 2)).reshape(C, 81).copy()  # [C, 81]
    iota = np.arange(C, dtype=np.float32)
    ema99 = (np.float32(0.99) * usage_ema).astype(np.float32)

    in_maps = []
    for core in range(N_CORES):
        xs = x[core * BS : (core + 1) * BS, 0]  # [BS, H, W]
        xpad = np.zeros((BS, HPS, WP), np.float32)
        xpad[:, 4 : 4 + H, 4 : 4 + W] = xs
        in_maps.append(
            {
                "xpad": xpad,
                "wenc": wenc,
                "wdn": wdn,
                "wup": wup,
                "wdec": wdec,
                "gate": gate_logit,
                "bdn": b_down,
                "bup": b_up,
                "iota": iota,
                "ema99": ema99,
            }
        )
    return in_maps


def kernel_nc():
    if "nc" not in _NC_CACHE:
        _NC_CACHE["nc"] = build_nc()
    return _NC_CACHE["nc"]


def kernel(x, w_conv, gate_logit, w_down, b_down, w_up, b_up, usage_ema):
    nc = kernel_nc()
    in_maps = make_in_maps(
        x, w_conv, gate_logit, w_down, b_down, w_up, b_up, usage_ema
    )
    res = run_bass_kernel_spmd(nc, in_maps, core_ids=list(range(N_CORES)))
    xhat = np.concatenate(
        [res.results[i]["xhat"].reshape(BS, 1, H, W) for i in range(N_CORES)], axis=0
    )
    usage = res.results[0]["usage"].reshape(C).copy()
    return xhat, usage


# revision 11
# speedup vs baseline: 1.0269x; 1.0269x over previous
"""Trainium2 Bass kernel for AEConv9 (9x9 conv autoencoder with top-1 channel
masking), data-parallel over batch across 8 NeuronCores.

Self-contained: hardcodes shapes from the problem spec
(x [16,1,128,128] f32, 256 channels, 128 latent, 9x9 kernels, pad 4).
"""

import sys

sys.path.insert(0, "/opt/trn_rl_repo")

import numpy as np

import concourse.bass as bass
import concourse.mybir as mybir
import concourse.tile as tile
from concourse.ap import AP
from concourse.masks import make_identity
from concourse.bass_utils import run_bass_kernel_spmd

F32 = mybir.dt.float32
F32R = mybir.dt.float32r
BF16 = mybir.dt.bfloat16
U32 = mybir.dt.uint32

N_CORES = 8
B = 16
BS = B // N_CORES  # images per core
C = 256  # channels
D = 128  # latent
H = W = 128
HP, WP = 136, 144  # padded image: 4 rows top/bottom, 4 cols left, 12 right
HPS = HP + 1  # +1 spare row so the flat 576-wide im2col window stays in bounds
NYS = HPS * WP
NCHUNK_PER_IMG = 32  # 4 rows of 128 px per chunk
F = 512  # pixels per chunk
NCHUNK = BS * NCHUNK_PER_IMG
RELU = mybir.ActivationFunctionType.Relu
SIGMOID = mybir.ActivationFunctionType.Sigmoid
COPYF = mybir.ActivationFunctionType.Copy


# ---------------------------------------------------------------- wait fixing
def _max_inline_waits(inst) -> int:
    return 0


def fix_waits(nc):
    """Hoist inline sem waits into standalone event-semaphore nops on the same
    engine queue (this walrus build supports ~0-1 inline waits per inst)."""
    counter = [0]

    def make_wait_nop(engine, wait):
        counter[0] += 1
        nop = mybir.InstEventSemaphore(name=f"WFIX-{counter[0]}")
        nop.engine = engine
        nop.sync_info = mybir.SyncInfo(on_wait=[wait], on_update=[])
        nc.register_instruction(nop, overwrite=True)
        return nop

    for f in nc.m.functions:
        for bb in f.blocks:
            insts = list(bb.instructions)
            out = []
            changed = False
            for inst in insts:
                si = inst.sync_info
                waits = list(si.on_wait) if (si and si.on_wait) else []
                keep = _max_inline_waits(inst)
                if len(waits) > keep:
                    changed = True
                    excess = waits[: len(waits) - keep]
                    kept = waits[len(waits) - keep :]
                    for w in excess:
                        out.append(make_wait_nop(inst.engine, w))
                    inst.sync_info.on_wait = kept
                out.append(inst)
            if changed:
                bb.instructions[:] = out
    return nc


# ---------------------------------------------------------------- builder
def build_nc(nchunk=NCHUNK):
    nc = bass.Bass(target_bir_lowering=False)

    xpad = nc.declare_dram_parameter("xpad", [BS, HPS, WP], F32, isOutput=False)
    wenc_d = nc.declare_dram_parameter("wenc", [81, C], F32, isOutput=False)
    wdn_d = nc.declare_dram_parameter("wdn", [C, D], F32, isOutput=False)
    wup_d = nc.declare_dram_parameter("wup", [D, C], F32, isOutput=False)
    wdec_d = nc.declare_dram_parameter("wdec", [C, 81], F32, isOutput=False)
    gate_d = nc.declare_dram_parameter("gate", [C], F32, isOutput=False)
    bdn_d = nc.declare_dram_parameter("bdn", [D], F32, isOutput=False)
    bup_d = nc.declare_dram_parameter("bup", [C], F32, isOutput=False)
    iota_d = nc.declare_dram_parameter("iota", [C], F32, isOutput=False)
    ema99_d = nc.declare_dram_parameter("ema99", [C], F32, isOutput=False)

    xhat_d = nc.declare_dram_parameter("xhat", [BS * H * W], F32, isOutput=True)
    usage_d = nc.declare_dram_parameter("usage", [C], F32, isOutput=True)

    # p scale: matches ref p = cnt / (B*H*W + 1e-6); usage = ema*0.99 + 0.01*p
    p_scale = float(np.float32(0.01) / np.float32(B * H * W + 1e-6))

    with tile.TileContext(nc) as tc:
        with (
            tc.tile_pool(name="const", bufs=1) as cpool,
            tc.tile_pool(name="work", bufs=1) as wpool,
            tc.tile_pool(name="psum", bufs=1, space="PSUM") as psp,
            tc.tile_pool(name="dram", bufs=1, space="DRAM") as dpool,
        ):
            # ---------------- constants / weights ----------------
            wenc = cpool.tile([81, C], F32, name="wenc")
            nc.sync.dma_start(wenc[:, :], wenc_d[:, :])

            # staging for f32r casts
            stg = cpool.tile([128, C], F32, name="stg")

            wdn = [cpool.tile([128, D], F32R, name=f"wdn{h}") for h in range(2)]
            nc.sync.dma_start(stg[:, 0:D], wdn_d[0:128, :])
            nc.vector.tensor_copy(wdn[0][:, :], stg[:, 0:D])
            nc.sync.dma_start(stg[:, D : 2 * D], wdn_d[128:256, :])
            nc.vector.tensor_copy(wdn[1][:, :], stg[:, D : 2 * D])

            wup = cpool.tile([D, C], F32R, name="wup")
            nc.sync.dma_start(stg[:, 0:C], wup_d[:, :])
            nc.vector.tensor_copy(wup[:, :], stg[:, 0:C])

            wdec = [cpool.tile([128, 81], F32R, name=f"wdec{h}") for h in range(2)]
            stg2 = cpool.tile([128, 192], F32, name="stg2")
            nc.sync.dma_start(stg2[:, 0:81], wdec_d[0:128, :])
            nc.vector.tensor_copy(wdec[0][:, :], stg2[:, 0:81])
            nc.sync.dma_start(stg2[:, 96 : 96 + 81], wdec_d[128:256, :])
            nc.vector.tensor_copy(wdec[1][:, :], stg2[:, 96 : 96 + 81])

            # small [128, 2] constants: col h = values for channels h*128..h*128+127
            def load_col2(dram):
                t = cpool.tile([128, 2], F32, name=f"cc_{dram.name}")
                src = AP(dram.ap().tensor, 0, [[1, 128], [128, 2]])
                nc.sync.dma_start(t[:, :], src)
                return t

            gate_t = load_col2(gate_d)
            iota_t = load_col2(iota_d)
            ema99_t = load_col2(ema99_d)
            bup_t = load_col2(bup_d)
            bdn_t = cpool.tile([128, 1], F32, name="bdn_t")
            nc.sync.dma_start(bdn_t[:, :], AP(bdn_d.ap().tensor, 0, [[1, 128], [128, 1]]))

            sig_t = cpool.tile([128, 2], F32, name="sig_t")
            nc.scalar.activation(sig_t[:, :], gate_t[:, :], SIGMOID)

            ident_f = cpool.tile([128, 128], F32, name="ident_f")
            make_identity(nc, ident_f[:, :])
            ident_b = cpool.tile([128, 128], BF16, name="ident_b")
            make_identity(nc, ident_b[:, :])
            ones_b = cpool.tile([1, 128], BF16, name="ones_b")
            nc.gpsimd.memset(ones_b[:, :], 1.0)
            ones81 = cpool.tile([81, 1], F32R, name="ones81")
            ones81f = cpool.tile([81, 1], F32, name="ones81f")
            nc.gpsimd.memset(ones81f[:, :], 1.0)
            nc.vector.tensor_copy(ones81[:, :], ones81f[:, :])

            # p-count accumulators
            pcnt = [cpool.tile([128, NCHUNK], F32, name=f"pcnt{h}") for h in range(2)]

            # DRAM Y staging, one per image (avoids cross-image WAR serialization)
            YSd = [dpool.tile([81, NYS], F32R, name=f"YSd{b}") for b in range(BS)]
            zero_sb = cpool.tile([81, 1536], F32R, name="zero_sb")
            zero_f = cpool.tile([81, 1536], F32, name="zero_f")
            nc.gpsimd.memset(zero_f[:, :], 0.0)
            nc.vector.tensor_copy(zero_sb[:, :], zero_f[:, :])
            for b in range(BS):
                ys3 = YSd[b][:, :].rearrange("p (y x) -> p y x", x=WP)
                # top 4 + bottom 4 (+1 spare) rows
                nc.sync.dma_start(ys3[:, 0:4, :], zero_sb[:, 0 : 4 * WP])
                nc.sync.dma_start(ys3[:, HP - 4 : HPS, :], zero_sb[:, 0 : 5 * WP])
                # left 4 and right 12 cols of interior rows
                nc.sync.dma_start(ys3[:, 4 : HP - 4, 0:4], zero_sb[:, 0 : 4 * (HP - 8)])
                nc.sync.dma_start(
                    ys3[:, 4 : HP - 4, WP - 12 : WP], zero_sb[:, 0 : 12 * (HP - 8)]
                )

            # ---------------- per-chunk pipeline ----------------
            state = {}

            def front(f):
                b, yb = divmod(f, NCHUNK_PER_IMG)
                y0 = 4 * yb
                # im2col load: XC[t=(dy,dx), q] = xpad[b, y0+dy, dx+q] flattened
                XC = wpool.tile([81, 576], F32, name="XC", bufs=4)
                src = AP(
                    xpad.ap().tensor,
                    b * NYS + y0 * WP,
                    [[WP, 9], [1, 9], [1, 576]],
                )
                nc.sync.dma_start(XC[:, :], src)
                xc3 = XC[:, :].rearrange("p (y x) -> p y x", x=WP)[:, :, 0:128]

                # encoder fp32 matmuls + gating
                A = []
                for h in range(2):
                    Z = psp.tile([128, F], F32, name=f"pz{h}", tag="pz", bufs=2)
                    nc.tensor.matmul(
                        Z[:, :].rearrange("p (y x) -> p y x", x=128),
                        wenc[:, h * 128 : (h + 1) * 128],
                        xc3,
                        start=True,
                        stop=True,
                    )
                    A_h = wpool.tile([128, F], F32, name=f"A{h}", bufs=3)
                    nc.scalar.activation(
                        A_h[:, :], Z[:, :], RELU, scale=sig_t[:, h : h + 1]
                    )
                    A.append(A_h)

                # per-subchunk argmax
                irow = psp.tile([1, F], F32, name="irow", tag="pmisc", bufs=2)
                for s in range(4):
                    TP = psp.tile([128, 256], F32, name="TP", tag="ptp", bufs=2)
                    for h in range(2):
                        nc.tensor.transpose(
                            TP[:, h * 128 : (h + 1) * 128],
                            A[h][:, s * 128 : (s + 1) * 128],
                            ident_f[:, :],
                        )
                    mx8 = wpool.tile([128, 8], F32, name="mx8", bufs=4)
                    idx8 = wpool.tile([128, 8], U32, name="idx8", bufs=4)
                    nc.vector.max(mx8[:, :], TP[:, :])
                    nc.vector.max_index(idx8[:, :], mx8[:, :], TP[:, :])
                    idxf = wpool.tile([128, 1], F32, name="idxf", bufs=4)
                    nc.vector.tensor_copy(idxf[:, :], idx8[:, 0:1])
                    nc.tensor.transpose(
                        irow[0:1, s * 128 : (s + 1) * 128], idxf[:, :], ident_f[:, :]
                    )
                irow_sb = wpool.tile([1, F], BF16, name="irow_sb", bufs=2)
                nc.scalar.copy(irow_sb[:, :], irow[:, :])
                IDXB = psp.tile([128, F], F32, name="IDXB", tag="pmisc", bufs=2)
                nc.tensor.matmul(
                    IDXB[:, :], ones_b[:, :], irow_sb[:, :], start=True, stop=True
                )

                # compare (counts) + fused apply -> masked activations in f32r
                Am = []
                for h in range(2):
                    mask_h = wpool.tile([128, F], BF16, name=f"mask{h}", bufs=2)
                    nc.vector.tensor_scalar(
                        out=mask_h[:, :],
                        in0=IDXB[:, :],
                        scalar1=iota_t[:, h : h + 1],
                        scalar2=0.0,
                        op0=mybir.AluOpType.is_equal,
                        op1=mybir.AluOpType.add,
                        accum_out=pcnt[h][:, f : f + 1],
                    )
                    Am_h = wpool.tile([128, F], F32R, name=f"Am{h}", bufs=3)
                    nc.vector.scalar_tensor_tensor(
                        out=Am_h[:, :],
                        in0=IDXB[:, :],
                        scalar=iota_t[:, h : h + 1],
                        in1=A[h][:, :],
                        op0=mybir.AluOpType.is_equal,
                        op1=mybir.AluOpType.mult,
                    )
                    Am.append(Am_h)

                # down projection (f32r)
                Hp = psp.tile([128, F], F32, name="Hp", tag="pmm", bufs=2)
                for h in range(2):
                    nc.tensor.matmul(
                        Hp[:, :], wdn[h][:, :], Am[h][:, :], start=(h == 0), stop=(h == 1)
                    )
                h_sb = wpool.tile([128, F], F32R, name="h_sb", bufs=3)
                nc.scalar.activation(h_sb[:, :], Hp[:, :], RELU, bias=bdn_t[:, :])

                # up projection
                A2 = []
                for h in range(2):
                    A2p = psp.tile([128, F], F32, name=f"A2p{h}", tag="pmm", bufs=2)
                    nc.tensor.matmul(
                        A2p[:, :], wup[:, h * 128 : (h + 1) * 128], h_sb[:, :],
                        start=True, stop=True,
                    )
                    a2_h = wpool.tile([128, F], F32R, name=f"a2_{h}", bufs=3)
                    nc.scalar.activation(
                        a2_h[:, :], A2p[:, :], RELU, bias=bup_t[:, h : h + 1]
                    )
                    A2.append(a2_h)

                # decoder Y = wdec^T @ a2  [81, F]
                Yp = psp.tile([81, F], F32, name="Yp", tag="pmm", bufs=2)
                for h in range(2):
                    nc.tensor.matmul(
                        Yp[:, :], wdec[h][:, :], A2[h][:, :], start=(h == 0), stop=(h == 1)
                    )
                Y_sb = wpool.tile([81, F], F32R, name="Y_sb", bufs=3)
                nc.scalar.copy(Y_sb[:, :], Yp[:, :])
                ys3 = YSd[b][:, :].rearrange("p (y x) -> p y x", x=WP)
                nc.sync.dma_start(ys3[:, y0 + 4 : y0 + 8, 4:132], Y_sb[:, :])

            def back(f):
                b, yb = divmod(f, NCHUNK_PER_IMG)
                y0 = 4 * yb
                G = wpool.tile([81, 576], F32R, name="G", bufs=4)
                gsrc = AP(
                    YSd[b][:, :].tensor,
                    y0 * WP,
                    [[NYS * 9 + WP, 9], [NYS + 1, 9], [1, 576]],
                )
                nc.sync.dma_start(G[:, :], gsrc)
                XHp = psp.tile([1, F], F32, name="XHp", tag="pmisc", bufs=2)
                g3 = G[:, :].rearrange("p (y x) -> p y x", x=WP)[:, :, 0:128]
                nc.tensor.matmul(
                    XHp[0:1, :].rearrange("p (y x) -> p y x", x=128),
                    ones81[:, :],
                    g3,
                    start=True,
                    stop=True,
                )
                xh_sb = wpool.tile([1, F], F32, name="xh_sb", bufs=2)
                nc.scalar.copy(xh_sb[:, :], XHp[:, :])
                nc.sync.dma_start(xhat_d[f * F : (f + 1) * F], xh_sb[0:1, :])

            for f in range(nchunk):
                front(f)
                if f >= 1:
                    back(f - 1)
            back(nchunk - 1)

            # ---------------- usage EMA with AllReduce ----------------
            cnt_sb = cpool.tile([128, 2], F32, name="cnt_sb")
            for h in range(2):
                nc.vector.reduce_sum(
                    cnt_sb[:, h : h + 1], pcnt[h][:, :], axis=mybir.AxisListType.X,
                    op=mybir.AluOpType.add,
                )
            pb_in = dpool.tile([C], F32, name="pb_in")
            pb_out = dpool.tile([C], F32, name="pb_out", addr_space="Shared")
            dst = AP(pb_in[:].tensor, 0, [[1, 128], [128, 2]])
            nc.gpsimd.dma_start(dst, cnt_sb[:, :])
            nc.gpsimd.collective_compute(
                "AllReduce",
                mybir.AluOpType.add,
                ins=[pb_in[:].opt()],
                outs=[pb_out[:].opt()],
                replica_groups=[list(range(N_CORES))],
            )
            cnt_g = cpool.tile([128, 2], F32, name="cnt_g")
            nc.gpsimd.dma_start(cnt_g[:, :], AP(pb_out[:].tensor, 0, [[1, 128], [128, 2]]))
            usage_sb = cpool.tile([128, 2], F32, name="usage_sb")
            nc.vector.scalar_tensor_tensor(
                out=usage_sb[:, :],
                in0=cnt_g[:, :],
                scalar=p_scale,
                in1=ema99_t[:, :],
                op0=mybir.AluOpType.mult,
                op1=mybir.AluOpType.add,
            )
            nc.gpsimd.dma_start(
                AP(usage_d.ap().tensor, 0, [[1, 128], [128, 2]]), usage_sb[:, :]
            )

    fix_waits(nc)
    return nc


_NC_CACHE = {}


def make_in_maps(x, w_conv, gate_logit, w_down, b_down, w_up, b_up, usage_ema):
    x = np.asarray(x, dtype=np.float32)
    w_conv = np.asarray(w_conv, dtype=np.float32)
    gate_logit = np.asarray(gate_logit, dtype=np.float32)
    w_down = np.asarray(w_down, dtype=np.float32)
    b_down = np.asarray(b_down, dtype=np.float32)
    w_up = np.asarray(w_up, dtype=np.float32)
    b_up = np.asarray(b_up, dtype=np.float32)
    usage_ema = np.asarray(usage_ema, dtype=np.float32)

    wenc = w_conv.reshape(C, 81).T.copy()  # [81, C], t = dy*9+dx
    wdn = w_down.T.copy()  # [C, D]
    wup = w_up.T.copy()  # [D, C]
    wdec = np.flip(w_conv[:, 0], axis=(1, 2)).reshape(C, 81).copy()  # [C, 81]
    iota = np.arange(C, dtype=np.float32)
    ema99 = (np.float32(0.99) * usage_ema).astype(np.float32)

    in_maps = []
    for core in range(N_CORES):
        xs = x[core * BS : (core + 1) * BS, 0]  # [BS, H, W]
        xpad = np.zeros((BS, HPS, WP), np.float32)
        xpad[:, 4 : 4 + H, 4 : 4 + W] = xs
        in_maps.append(
            {
                "xpad": xpad,
                "wenc": wenc,
                "wdn": wdn,
                "wup": wup,
                "wdec": wdec,
                "gate": gate_logit,
                "bdn": b_down,
                "bup": b_up,
                "iota": iota,
                "ema99": ema99,
            }
        )
    return in_maps


def kernel_nc():
    if "nc" not in _NC_CACHE:
        _NC_CACHE["nc"] = build_nc()
    return _NC_CACHE["nc"]


def kernel(x, w_conv, gate_logit, w_down, b_down, w_up, b_up, usage_ema):
    nc = kernel_nc()
    in_maps = make_in_maps(
        x, w_conv, gate_logit, w_down, b_down, w_up, b_up, usage_ema
    )
    res = run_bass_kernel_spmd(nc, in_maps, core_ids=list(range(N_CORES)))
    xhat = np.concatenate(
        [res.results[i]["xhat"].reshape(BS, 1, H, W) for i in range(N_CORES)], axis=0
    )
    usage = res.results[0]["usage"].reshape(C).copy()
    return xhat, usage
